# revision 1
# baseline (speedup 1.0000x reference)
"""Matrix-Tree edge marginals on 8 Trainium2 NeuronCores.

probs[b,i,j] = d logZ / d scores[b,i,j] with logZ from the Matrix-Tree
theorem.  Closed form: with A = exp(masked scores - m) and Lfull the
(row/col-0-padded) Laplacian, probs = A ⊙ (diag(Y)·1^T − Y) where
Y = (Lfull^T)^{-1}.  We invert M = Lfull^T by Newton-Schulz on device
(9 bf16 iters, maintaining the pair Y, W = Y^T so no transposes are
needed), then 2 rounds of f32 residual refinement with the stale bf16
W as left factor.  Batch (256) is sharded 32 per core; matrices are
processed in interleaved groups of 6, with the next group's setup
software-pipelined into the current group's iterations, keeping TensorE
dense and warm.
"""

import numpy as np

import concourse.bass as bass
import concourse.bacc as bacc
import concourse.mybir as mybir
from concourse.bass import ds, ts
from concourse.masks import make_identity
from concourse.tile import TileContext
from concourse.bass_utils import run_bass_kernel_spmd

B, S, P = 256, 256, 128
NCORES = 8
BPC = B // NCORES   # matrices per core
RB = S // P         # row blocks per matrix
NB_ITERS = 9        # bf16 Newton-Schulz iterations
NR_ROUNDS = 2       # f32 residual-refinement rounds
GRP = 6             # matrices interleaved per group
NEG = np.float32(-1e9)

f32 = mybir.dt.float32
bf16 = mybir.dt.bfloat16
MULT = mybir.AluOpType.mult
ADD = mybir.AluOpType.add
SUB = mybir.AluOpType.subtract
AX = mybir.AxisListType.X
COPY = mybir.ActivationFunctionType.Copy
EXP = mybir.ActivationFunctionType.Exp

PACK = RB * S + 3  # per-partition packed input: scores rows | rmask | negm


def _mm256(nc, out_ps, lhsT, rhs):
    """256x256 product into PSUM: for each output row-block I accumulate
    over contraction blocks K.  out = (stored lhsT)^T @ rhs."""
    for I in range(RB):
        for K in range(RB):
            nc.tensor.matmul(
                out_ps[:, I, :],
                lhsT[:, K, ts(I, P)],
                rhs[:, K, :],
                start=(K == 0),
                stop=(K == RB - 1),
            )


def build_program():
    nc = bacc.Bacc()
    inp = nc.dram_tensor("inp", [BPC, P, PACK], f32, kind="ExternalInput")
    out = nc.dram_tensor("probs", [BPC, S, S], f32, kind="ExternalOutput")

    with TileContext(nc) as tc:
        with (
            tc.tile_pool(name="consts", bufs=1) as consts,
            tc.tile_pool(name="mat", bufs=8) as mat,
            tc.tile_pool(name="small", bufs=8) as small,
            tc.tile_pool(name="it", bufs=14) as itp,
            tc.tile_pool(name="psum", bufs=4, space="PSUM") as pp,
            tc.tile_pool(name="psumYW", bufs=2, space="PSUM") as ppyw,
        ):
            ident = consts.tile([P, P], f32)
            make_identity(nc, ident)
            # full-size I and 2I: U = 2I - T and R = I - T each one DVE op
            identbig = consts.tile([P, RB, S], f32)
            nc.vector.memset(identbig, 0.0)
            for rb in range(RB):
                nc.vector.tensor_copy(identbig[:, rb, ts(rb, P)], ident)
            ident2big = consts.tile([P, RB, S], f32)
            nc.vector.tensor_scalar_mul(ident2big, identbig, 2.0)

            def setup(b):
                st = {}
                packed = mat.tile([P, PACK], f32, tag="packed", bufs=12)
                nc.sync.dma_start(packed, inp[b])
                Sp = packed[:, : RB * S].rearrange("p (rb j) -> p rb j", rb=RB)
                negm = packed[:, RB * S + 2 : RB * S + 3]
                rf = small.tile([P, RB], f32, tag="rf")
                nc.scalar.activation(rf, packed[:, RB * S : RB * S + 2], COPY)

                # A = exp(s - m); unmasked/row0 entries underflow to 0
                Aa = mat.tile([P, RB, S], f32, tag="Aa", bufs=13)
                nc.scalar.activation(Aa, Sp, EXP, bias=negm, scale=1.0)
                st["Aa"] = Aa

                d = small.tile([P, RB], f32, tag="d")
                nc.vector.tensor_reduce(d, Aa, AX, ADD)
                nrf = small.tile([P, RB], f32, tag="nrf")
                nc.vector.tensor_scalar(
                    out=nrf, in0=rf, scalar1=-1.0, scalar2=1.0, op0=MULT, op1=ADD
                )
                negrf = small.tile([P, RB], f32, tag="negrf")
                nc.vector.tensor_scalar_mul(negrf, rf, -1.0)
                c1 = small.tile([P, RB], f32, tag="c1")
                nc.vector.tensor_mul(c1, d, rf)
                nc.vector.tensor_add(c1, c1, nrf)

                L = mat.tile([P, RB, S], f32, tag="L", bufs=13)
                for rb in range(RB):
                    for jb in range(RB):
                        blk = L[:, rb, ts(jb, P)]
                        if jb != rb:
                            nc.vector.tensor_scalar_mul(
                                blk, Aa[:, rb, ts(jb, P)], negrf[:, ds(rb, 1)]
                            )
                        else:
                            tmp = small.tile([P, P], f32, tag="tmp")
                            nc.vector.tensor_scalar_mul(tmp, ident, c1[:, ds(rb, 1)])
                            nc.vector.tensor_scalar_mul(
                                blk, Aa[:, rb, ts(jb, P)], rf[:, ds(rb, 1)]
                            )
                            nc.vector.tensor_sub(blk, tmp, blk)
                nc.vector.memset(L[:, :, 0:1], 0.0)
                nc.vector.memset(L[0:1, 0, :], 0.0)
                nc.vector.memset(L[0:1, 0, 0:1], 1.0)
                st["L"] = L

                Lbf = mat.tile([P, RB, S], bf16, tag="Lbf", bufs=13)
                nc.scalar.activation(Lbf, L, COPY)
                st["Lbf"] = Lbf

                # Jacobi init Y0 = W0 = diag(1/diag(L))
                dl = small.tile([P, RB], f32, tag="dl")
                for rb in range(RB):
                    ttrd = small.tile([P, P], f32, tag="ttrd")
                    nc.vector.tensor_mul(ttrd, ident, L[:, rb, ts(rb, P)])
                    nc.vector.tensor_reduce(dl[:, ds(rb, 1)], ttrd, AX, ADD)
                r0 = small.tile([P, RB], f32, tag="r0")
                nc.vector.reciprocal(r0, dl)

                YW = itp.tile([P, RB, 2 * S], bf16, tag="YW0", bufs=8)
                nc.vector.memset(YW, 0.0)
                for rb in range(RB):
                    nc.vector.tensor_scalar_mul(
                        YW[:, rb, ts(rb, P)], ident, r0[:, ds(rb, 1)]
                    )
                    nc.vector.tensor_scalar_mul(
                        YW[:, rb, ds(S + rb * P, P)], ident, r0[:, ds(rb, 1)]
                    )
                st["YW"] = YW
                return st

            def newton_step(st):
                YWc = st["YW"]
                Yc = YWc[:, :, 0:S]
                Wc = YWc[:, :, S : 2 * S]
                T1 = pp.tile([P, RB, S], f32, tag="ps")
                _mm256(nc, T1, st["Lbf"], Yc)          # T1 = L^T @ Y = M Y
                U = itp.tile([P, RB, S], bf16, tag="U")
                nc.vector.tensor_sub(U, ident2big, T1)
                YWp = ppyw.tile([P, RB, 2 * S], f32, tag="psYW")
                _mm256(nc, YWp[:, :, 0:S], Wc, U)      # Y@U
                _mm256(nc, YWp[:, :, S : 2 * S], U, Wc)  # (Y@U)^T
                YW = itp.tile([P, RB, 2 * S], bf16, tag="YW")
                nc.scalar.activation(YW, YWp, COPY)
                st["YW"] = YW

            def promote(st):
                Yf = mat.tile([P, RB, S], f32, tag="Yf")
                nc.scalar.activation(Yf, st["YW"][:, :, 0:S], COPY)
                st["Yf"] = Yf

            def refine_round(st):
                T1 = pp.tile([P, RB, S], f32, tag="ps")
                _mm256(nc, T1, st["L"], st["Yf"])      # f32: M @ Yf
                Rr = itp.tile([P, RB, S], bf16, tag="U")
                nc.vector.tensor_sub(Rr, identbig, T1)  # residual I - M Yf
                dYp = pp.tile([P, RB, S], f32, tag="ps")
                _mm256(nc, dYp, st["YW"][:, :, S : 2 * S], Rr)  # stale-W: Y @ R
                Yf = mat.tile([P, RB, S], f32, tag="Yf")
                nc.vector.tensor_add(Yf, st["Yf"], dYp)
                st["Yf"] = Yf

            def output(b, st):
                Yf, Aa = st["Yf"], st["Aa"]
                dY = small.tile([P, RB], f32, tag="dY")
                for rb in range(RB):
                    ttrd = small.tile([P, P], f32, tag="ttrd")
                    nc.vector.tensor_mul(ttrd, ident, Yf[:, rb, ts(rb, P)])
                    nc.vector.tensor_reduce(dY[:, ds(rb, 1)], ttrd, AX, ADD)
                Pr = mat.tile([P, RB, S], f32, tag="Pr")
                for rb in range(RB):
                    nc.vector.tensor_scalar(
                        out=Pr[:, rb, :],
                        in0=Yf[:, rb, :],
                        scalar1=dY[:, ds(rb, 1)],
                        scalar2=-1.0,
                        op0=SUB,
                        op1=MULT,
                    )
                nc.vector.tensor_mul(Pr, Pr, Aa)
                nc.sync.dma_start(
                    out[b].rearrange("(rb p) j -> p rb j", p=P), Pr
                )

            groups = [
                list(range(g0, min(g0 + GRP, BPC)))
                for g0 in range(0, BPC, GRP)
            ]
            sts = {}
            for gi, grp in enumerate(groups):
                if gi == 0:
                    for b in grp:
                        sts[b] = setup(b)
                nxt = groups[gi + 1] if gi + 1 < len(groups) else []
                for k in range(NB_ITERS):
                    for b in grp:
                        newton_step(sts[b])
                    if k < len(nxt):
                        sts[nxt[k]] = setup(nxt[k])
                for b in grp:
                    promote(sts[b])
                for r in range(NR_ROUNDS):
                    for b in grp:
                        refine_round(sts[b])
                for b in grp:
                    output(b, sts[b])
    nc.finalize()
    return nc


_prog = None


def _get_program():
    global _prog
    if _prog is None:
        _prog = build_program()
    return _prog


def _host_prep(scores, mask):
    scores = np.asarray(scores, dtype=np.float32)
    mask = np.asarray(mask).astype(bool)
    mr = mask.copy()
    mr[:, 0] = True
    pair = mr[:, :, None] & mr[:, None, :]
    spre = np.where(pair, scores, NEG)
    spre[:, 0, :] = NEG
    m = spre.max(axis=(1, 2))                      # [B]
    packed = np.empty((B, P, PACK), dtype=np.float32)
    packed[:, :, : RB * S] = (
        spre.reshape(B, RB, P, S).transpose(0, 2, 1, 3).reshape(B, P, RB * S)
    )
    packed[:, :, RB * S : RB * S + 2] = (
        mr.astype(np.float32).reshape(B, RB, P).transpose(0, 2, 1)
    )
    packed[:, :, RB * S + 2] = (-m)[:, None]
    return packed


def kernel(scores, mask):
    packed = _host_prep(scores, mask)
    nc = _get_program()
    in_maps = [
        {"inp": packed[i * BPC:(i + 1) * BPC]}
        for i in range(NCORES)
    ]
    res = run_bass_kernel_spmd(nc, in_maps, list(range(NCORES)))
    return np.concatenate(
        [res.results[i]["probs"] for i in range(NCORES)], axis=0
    ).astype(np.float32)

